# revision 31
# baseline (speedup 1.0000x reference)
"""Distributed Trainium2 kernel for quantized attention (nn_Attention_25812753449411).

Sharding: 16 heads across 8 cores (2 heads/core), batch-of-heads parallel —
no collectives. Host prepares dequantized fp16/bf16 operands per head (k
permuted so a stride-8 sample forms the first 256 columns).

Per qtile (128 q rows) the device computes QK^T in fp16 (f32 accum) into two
PSUM pieces: A = cols 0:512 (contains the sample), B = cols 512:2048. A
sampled row-max on DVE gives the exp bias, then the two pieces exponentiate
on different engines so no single engine is the bottleneck:
  - piece B (1536 cols): ScalarE exp with fused per-row accumulation;
  - piece A (512 cols): DVE tensor_scalar applies the bias (PSUM f32 ->
    SBUF bf16), GpSimd tensor_tensor computes pow(e, s') (pow is only
    ISA-legal on Pool), and DVE sums rows with a 4x-mode tensor_scalar
    (deferred one slot so the in-order DVE queue never waits on Pool).
The loop is software-pipelined one qtile deep: slot i runs QK.A and the
sampled reduce_max for qtile i+1, so the serial chain matmul->max->exp never
gates the ACT engine. P (bf16) transposes on-chip via the DMA xbar per
qtile; P^T@V runs in ctx^T orientation through a work queue interleaved
with QK^T. The DMA engine is the modeled bottleneck (~2us/qtile transpose +
all loads/stores on one serial device), so h1's input loads are interleaved
into early h0 slots and ctx stores are emitted in 512-col quarters.

Math: scores = Qdeq @ Kdeq^T / sqrt(d). softmax is shift-invariant, so any
per-row m_hat <= max works as the exp bias while exp(max - m_hat) stays
finite in f32/bf16; rows where the stride-8 sample underestimates the max by
more than ~85 overflow to inf, are detected on the host, and recomputed
exactly there (typically <2% of rows).
"""

import sys

sys.path.insert(0, "/opt/trn_rl_repo")

import numpy as np
import ml_dtypes

S, B, H, D = 2048, 1, 16, 128
N_CORES = 8
HPC = H // N_CORES  # heads per core
QT = S // 128  # q tiles per head
KT = S // 128  # k tiles per head
NSLOT = HPC * QT  # qtile slots per core
NSAMP = 128  # stride-16 sample columns (permuted to the front)
WA = 512  # piece A width (DVE+Pool path, holds the sample)
WB = S - WA  # piece B width (ACT path)
KTA = WA // 128  # kt tiles covered by piece A
# (head, qtile) pairs whose P transposes on DVE+Pool instead of the DMA xbar
# (measured: the offload's pipeline coupling costs more than the serial-DMA
# time it saves, so it is disabled)
DVE_TR = set()

_compiled = None


def _build_graph(do_compile=True):
    import concourse.mybir as mybir
    import concourse.tile as tile
    from concourse import bacc

    fp16 = mybir.dt.float16
    bf16 = mybir.dt.bfloat16
    f32 = mybir.dt.float32
    Exp = mybir.ActivationFunctionType.Exp
    Alu = mybir.AluOpType
    AxX = mybir.AxisListType.X

    nc = bacc.Bacc()

    qdT = nc.declare_dram_parameter("qdT", [HPC, 128, S], fp16, isOutput=False)
    kdT = nc.declare_dram_parameter("kdT", [HPC, 128, S], fp16, isOutput=False)
    vd = nc.declare_dram_parameter("vd", [HPC, 128, KT, 128], bf16, isOutput=False)
    ctxT = nc.declare_dram_parameter("ctxT", [HPC, 128, S], bf16, isOutput=True)
    lsum = nc.declare_dram_parameter("lsum", [HPC, 128, 2 * QT], f32, isOutput=True)

    with tile.TileContext(nc) as tc:
        with (
            tc.tile_pool(name="ins", bufs=1) as ins_pool,
            tc.tile_pool(name="p", bufs=4) as p_pool,
            tc.tile_pool(name="pt", bufs=4) as pt_pool,
            tc.tile_pool(name="sp", bufs=2) as sp_pool,
            tc.tile_pool(name="scr", bufs=2) as scr_pool,
            tc.tile_pool(name="st", bufs=2) as st_pool,
            tc.tile_pool(name="stat", bufs=6) as stat_pool,
            tc.tile_pool(name="lout", bufs=1) as lout_pool,
            tc.tile_pool(name="cout", bufs=2) as cout_pool,
            tc.tile_pool(name="sA", bufs=1, space="PSUM") as sA_pool,
            tc.tile_pool(name="sB", bufs=2, space="PSUM") as sB_pool,
            tc.tile_pool(name="c", bufs=1, space="PSUM") as c_pool,
        ):
            heads = []
            for h in range(HPC):
                qdT_s = ins_pool.tile([128, S], fp16, tag=f"qdT{h}")
                kdT_s = ins_pool.tile([128, S], fp16, tag=f"kdT{h}")
                vd_s = ins_pool.tile([128, KT, 128], bf16, tag=f"vd{h}")
                l_s = lout_pool.tile([128, 2 * QT], f32, tag=f"l{h}")
                heads.append((qdT_s, kdT_s, vd_s, l_s))

            # h0 loads up front, fine-grained so qt0's first matmul starts
            # ASAP; h1 loads are interleaved into early h0 slots (the DMA
            # engine is serial — front-loading them would delay transposes)
            qdT_s, kdT_s, vd_s, _ = heads[0]
            nc.sync.dma_start(out=kdT_s[:, 0:512], in_=kdT[0][:, 0:512])
            nc.sync.dma_start(out=qdT_s[:, 0:128], in_=qdT[0][:, 0:128])
            nc.sync.dma_start(out=kdT_s[:, 512:], in_=kdT[0][:, 512:])
            nc.sync.dma_start(out=qdT_s[:, 128:], in_=qdT[0][:, 128:])
            nc.sync.dma_start(out=vd_s[:], in_=vd[0])
            h1_loads = []
            if HPC > 1:
                q1, k1, v1, _ = heads[1]
                h1_loads = [
                    lambda: nc.sync.dma_start(out=k1[:, 0:1024], in_=kdT[1][:, 0:1024]),
                    lambda: nc.sync.dma_start(out=k1[:, 1024:], in_=kdT[1][:, 1024:]),
                    lambda: nc.sync.dma_start(out=q1[:, 0:1024], in_=qdT[1][:, 0:1024]),
                    lambda: nc.sync.dma_start(out=q1[:, 1024:], in_=qdT[1][:, 1024:]),
                    lambda: nc.sync.dma_start(out=v1[:, 0:8, :], in_=vd[1][:, 0:8, :]),
                    lambda: nc.sync.dma_start(out=v1[:, 8:, :], in_=vd[1][:, 8:, :]),
                ]

            # e-base table for the GpSimd pow path (pow(e, s') == exp(s'))
            e_s = ins_pool.tile([128, WA], bf16, tag="e")
            nc.gpsimd.memset(e_s[:], float(np.e))

            # warm the ACT exp table while input DMAs run (reads garbage,
            # result discarded — only the implicit table load matters)
            warm = stat_pool.tile([128, 1], f32, tag="warm")
            nc.scalar.activation(warm[:], warm[:], Exp)

            head_ctx = [cout_pool.tile([128, S], bf16, tag="ctx", name=f"ctx{h}")
                        for h in range(HPC)]

            pt_tiles = {}
            ctx_ps = {}
            quarters_stored = [0 for _ in range(HPC)]
            pairs_copied = [0 for _ in range(HPC)]

            def pt_tile(h, pr):
                if (h, pr) not in pt_tiles:
                    pt_tiles[(h, pr)] = pt_pool.tile(
                        [128, 2, KT, 128], bf16, tag="pt", name=f"pt{h}_{pr}"
                    )
                return pt_tiles[(h, pr)]

            def get_ctx_ps(h, pr):
                if (h, pr) not in ctx_ps:
                    ctx_ps[(h, pr)] = c_pool.tile(
                        [128, 256], f32, tag="c", name=f"cps{h}_{pr}",
                        padded_shape=[128, 512],
                    )
                return ctx_ps[(h, pr)]

            def finish_pair(h, pr, cps):
                ctx_s = head_ctx[h]
                nc.vector.tensor_copy(
                    out=ctx_s[:, pr * 256 : (pr + 1) * 256], in_=cps[:]
                )
                del ctx_ps[(h, pr)]
                pairs_copied[h] += 1
                done = pairs_copied[h]
                last_head = h == HPC - 1
                if last_head and done == 7:
                    # store eagerly so only a 256-col store trails the last PV
                    nc.sync.dma_start(
                        out=ctxT[h][:, 1536:1792], in_=ctx_s[:, 1536:1792]
                    )
                elif last_head and done == 8:
                    nc.sync.dma_start(
                        out=ctxT[h][:, 1792:2048], in_=ctx_s[:, 1792:2048]
                    )
                elif done % 2 == 0:
                    qtr = quarters_stored[h]
                    nc.sync.dma_start(
                        out=ctxT[h][:, qtr * 512 : (qtr + 1) * 512],
                        in_=ctx_s[:, qtr * 512 : (qtr + 1) * 512],
                    )
                    quarters_stored[h] += 1

            # PV work queue:
            #   (ready, h, pr, half)            pair entry: 8 kt over 256 q
            #   (ready, h, pr, (i, part))       tail entry: kt 0:4 / 4:16,
            #                                   single 128-q qtile
            pv_queue = []
            singles_done = {}

            def emit_pv(ent):
                _, h, pr, half = ent
                _, _, vd_s, _ = heads[h]
                pt_s = pt_tiles[(h, pr)]
                cps = get_ctx_ps(h, pr)
                if isinstance(half, tuple):
                    i, part = half
                    kts = range(0, KTA) if part == 0 else range(KTA, KT)
                    for kt in kts:
                        nc.tensor.matmul(
                            cps[:, i * 128 : (i + 1) * 128],
                            vd_s[:, kt, :],
                            pt_s[:, i, kt, :],
                            start=(kt == 0),
                            stop=(kt == KT - 1),
                        )
                    if part == 1:
                        singles_done[(h, pr)] = singles_done.get((h, pr), 0) + 1
                        if singles_done[(h, pr)] == 2:
                            finish_pair(h, pr, cps)
                    return
                for kt in range(8 * half, 8 * half + 8):
                    nc.tensor.matmul(
                        cps[:],
                        vd_s[:, kt, :],
                        pt_s[:, :, kt, :],
                        start=(kt == 0),
                        stop=(kt == KT - 1),
                    )
                if half == 1:
                    finish_pair(h, pr, cps)

            def drain_pv(gqt, n=2):
                for _ in range(n):
                    if pv_queue and pv_queue[0][0] <= gqt:
                        emit_pv(pv_queue.pop(0))

            sA_t = {}
            nm_t = {}

            def emit_qkA(gqt):
                """QK^T piece A + sampled row-max for slot gqt (pipelined one
                slot ahead)."""
                h, qt = divmod(gqt, QT)
                qdT_s, kdT_s, _, _ = heads[h]
                lhs = qdT_s[:, qt * 128 : (qt + 1) * 128]
                sA = sA_pool.tile([128, WA], f32, tag="sA")
                nc.tensor.matmul(sA[:], lhs, kdT_s[:, 0:WA], start=True, stop=True)
                nm = stat_pool.tile([128, 1], f32, tag="m")
                nc.vector.reduce_max(nm[:], sA[:, 0:NSAMP], axis=AxX, negate=True)
                sA_t[gqt], nm_t[gqt] = sA, nm

            pending_accum = []

            def flush_accum():
                while pending_accum:
                    pout, lcol = pending_accum.pop(0)
                    scr = scr_pool.tile([128, WA], bf16, tag="scr")
                    nc.vector.tensor_scalar(
                        scr[:], pout, 1.0, None, Alu.mult, Alu.add,
                        accum_out=lcol,
                    )

            # DVE-side transpose: StreamTranspose 32x32 blocks (4 split
            # pieces, 2 per slot), then 16 strided 4x-mode copies permute the
            # blocks into PT layout (metered out 6 per slot, alternating
            # DVE/Pool so neither engine's in-order queue backs up)
            st_jobs = []

            def pump_st():
                if not st_jobs:
                    return
                job = st_jobs[0]
                if job["pieces"]:
                    for _ in range(2):
                        if job["pieces"]:
                            p0 = job["pieces"].pop(0)
                            nc.vector.transpose(
                                out=job["st"][:, 512 * p0 : 512 * (p0 + 1)],
                                in_=job["src"][:, 512 * p0 : 512 * (p0 + 1)],
                            )
                    return
                for _ in range(6):
                    if not job["copies"]:
                        break
                    qh, m = job["copies"].pop(0)
                    eng = nc.vector if (qh + m) % 2 == 0 else nc.gpsimd
                    eng.tensor_copy(
                        out=job["pt"][32 * m : 32 * m + 32, job["i"], :,
                                      32 * qh : 32 * qh + 32],
                        in_=job["st"].rearrange("p (t x) -> p t x", x=128)[
                            32 * qh : 32 * qh + 32, :, 32 * m : 32 * m + 32
                        ],
                    )
                if not job["copies"]:
                    st_jobs.pop(0)

            def flush_st():
                while st_jobs:
                    pump_st()

            emit_qkA(0)  # prologue
            cur_pair = None
            for gqt in range(NSLOT):
                h, qt = divmod(gqt, QT)
                qdT_s, kdT_s, vd_s, l_s = heads[h]
                pr, i = divmod(qt, 2)
                if i == 0:
                    cur_pair = p_pool.tile(
                        [128, 2, S], bf16, tag="p", name=f"p{h}_{pr}"
                    )
                p_pair = cur_pair
                lhs = qdT_s[:, qt * 128 : (qt + 1) * 128]
                sA, nm = sA_t.pop(gqt), nm_t.pop(gqt)

                drain_pv(gqt)

                # --- piece B matmuls ---
                sB = sB_pool.tile([128, WB], f32, tag="sB")
                for j in range(3):
                    nc.tensor.matmul(
                        sB[:, j * 512 : (j + 1) * 512],
                        lhs,
                        kdT_s[:, WA + j * 512 : WA + (j + 1) * 512],
                        start=True, stop=True,
                    )

                # --- piece A: DVE bias -> Pool pow ---
                poutA = p_pair[:, i, 0:WA]
                spA = sp_pool.tile([128, WA], bf16, tag="sp")
                nc.vector.tensor_scalar(spA[:], sA[:], nm[:], None, Alu.add)
                nc.gpsimd.tensor_tensor(
                    out=poutA, in0=e_s[:], in1=spA[:], op=Alu.pow
                )
                # --- piece B: ACT exp with fused accum ---
                poutB = p_pair[:, i, WA:S]
                nc.scalar.activation(
                    poutB, sB[:], Exp, bias=nm[:], scale=1.0,
                    accum_out=l_s[:, 2 * qt + 1 : 2 * qt + 2],
                )

                # software pipeline: next slot's piece A + row-max (sA tile
                # was released by this slot's tensor_scalar)
                if gqt + 1 < NSLOT:
                    emit_qkA(gqt + 1)

                # deferred row-sum for the previous slot's piece A (its Pool
                # pow has long finished — no in-order DVE stall)
                if qt == QT - 1:
                    pending_accum.append((poutA, l_s[:, 2 * qt : 2 * qt + 1]))
                    flush_accum()
                    nc.sync.dma_start(out=lsum[h], in_=l_s[:])
                else:
                    flush_accum()
                    pending_accum.append((poutA, l_s[:, 2 * qt : 2 * qt + 1]))

                # per-qtile transpose: most qtiles via the DMA xbar (serial
                # DMA device — the modeled bottleneck), a few via DVE to
                # offload it; PV pair entries become ready a couple slots on
                if h == HPC - 1 and pr == QT // 2 - 1:
                    # tail: per-piece transposes + per-qtile PV entries keep
                    # the post-loop chain short
                    nc.sync.dma_start_transpose(
                        out=pt_tile(h, pr)[:, i, 0:KTA, :], in_=poutA
                    )
                    nc.sync.dma_start_transpose(
                        out=pt_tile(h, pr)[:, i, KTA:KT, :], in_=poutB
                    )
                    pv_queue.append((gqt + 1, h, pr, (i, 0)))
                    pv_queue.append((gqt + 1, h, pr, (i, 1)))
                    pv_queue.sort(key=lambda t: t[0])
                elif (h, qt) in DVE_TR:
                    st_jobs.append({
                        "st": st_pool.tile([128, S], bf16, tag="st",
                                           name=f"st{h}_{qt}"),
                        "src": p_pair[:, i, :],
                        "pt": pt_tile(h, pr),
                        "i": i,
                        "pieces": [0, 1, 2, 3],
                        "copies": [(qh, m) for qh in range(4) for m in range(4)],
                    })
                else:
                    nc.sync.dma_start_transpose(
                        out=pt_tile(h, pr)[:, i, :, :], in_=p_pair[:, i, :]
                    )
                if i == 1 and not (h == HPC - 1 and pr == QT // 2 - 1):
                    rdy = gqt + 2
                    if (h, qt) in DVE_TR or (h, qt - 1) in DVE_TR:
                        rdy += 5
                    pv_queue.append((rdy, h, pr, 0))
                    pv_queue.append((rdy, h, pr, 1))
                    pv_queue.sort(key=lambda t: t[0])
                pump_st()
                if h == 0 and h1_loads and qt in (1, 3, 5, 7, 9, 11):
                    h1_loads.pop(0)()

            flush_st()
            while pv_queue:
                emit_pv(pv_queue.pop(0))
    if do_compile:
        nc.compile()
    return nc


def _get_compiled():
    global _compiled
    if _compiled is None:
        _compiled = _build_graph()
    return _compiled


def _prep_core_inputs(c, QdT, KdT, Vd):
    """Slice per-core head shards. QdT/KdT: [H,128,S] f16, Vd: [H,128,KT,128] bf16."""
    hs = slice(c * HPC, (c + 1) * HPC)
    return {
        "qdT": np.ascontiguousarray(QdT[hs]),
        "kdT": np.ascontiguousarray(KdT[hs]),
        "vd": np.ascontiguousarray(Vd[hs]),
    }


def kernel(q, k, v, qmin, qscale, kmin, kscale, vmin, vscale, _trace=False):
    from concourse.bass_utils import run_bass_kernel_spmd

    f32 = np.float32
    q, k, v = np.asarray(q), np.asarray(k), np.asarray(v)
    qmin, qscale = np.asarray(qmin), np.asarray(qscale)
    kmin, kscale = np.asarray(kmin), np.asarray(kscale)
    vmin, vscale = np.asarray(vmin), np.asarray(vscale)
    # [S,B,H,D] -> [H,S,D]
    qh = np.transpose(q.astype(f32), (1, 2, 0, 3))[0]
    kh = np.transpose(k.astype(f32), (1, 2, 0, 3))[0]
    vh = np.transpose(v.astype(f32), (1, 2, 0, 3))[0]

    def col(x):  # [S,B,H,1] -> [H,S,1]
        return np.transpose(x.astype(f32), (1, 2, 0, 3))[0]

    qs, qm = col(qscale), col(qmin)
    ks, km = col(kscale), col(kmin)
    vs, vm = col(vscale), col(vmin)

    inv_sqrt_d = 1.0 / np.sqrt(np.float32(D))
    Qd = (qs * qh + qm) * inv_sqrt_d          # [H,S,D] f32
    Kd = ks * kh + km
    Vd = vs * vh + vm

    QdT = np.ascontiguousarray(Qd.transpose(0, 2, 1)).astype(np.float16)   # [H,128,S]
    # permute k so the stride-16 sample (submax source) is the first 128 columns
    perm = np.concatenate([np.arange(0, S, 16),
                           np.setdiff1d(np.arange(S), np.arange(0, S, 16))])
    KdT = np.ascontiguousarray(Kd.transpose(0, 2, 1)[:, :, perm]).astype(np.float16)
    Vdp = Vd[:, perm, :]
    # Vd [H,S,D] -> [H, k_in(128), kt(KT), d(128)]
    Vd4 = np.ascontiguousarray(
        Vdp.reshape(H, KT, 128, D).transpose(0, 2, 1, 3)
    ).astype(ml_dtypes.bfloat16)

    nc = _get_compiled()
    in_maps = [_prep_core_inputs(c, QdT, KdT, Vd4) for c in range(N_CORES)]
    try:
        res = run_bass_kernel_spmd(nc, in_maps, list(range(N_CORES)), trace=_trace)
    except Exception:
        # transient NRT device errors have been observed once; retry once
        res = run_bass_kernel_spmd(nc, in_maps, list(range(N_CORES)), trace=_trace)
    results = res.results

    out = np.zeros((S, B, H * D), np.float32)
    with np.errstate(invalid="ignore", over="ignore", divide="ignore"):
        for c in range(N_CORES):
            for i in range(HPC):
                h = c * HPC + i
                ctxT_un = results[c]["ctxT"][i].astype(f32)      # [128(d), S(q)]
                ls = results[c]["lsum"][i].astype(f32)           # [128, 2*QT]
                l_full = (ls[:, 0::2] + ls[:, 1::2]).T.reshape(S)  # q = qt*128 + p
                ctx = ctxT_un.T / l_full[:, None]
                # Rows where the sampled max underestimates the true max by
                # >~85 overflow to inf somewhere; recompute exactly in f32.
                bad = ~np.isfinite(ctx).all(1) | ~np.isfinite(l_full)
                if bad.any():
                    rows = np.where(bad)[0]
                    Srow = (Qd[h][rows] @ Kd[h].T)               # [n, S] f32
                    Srow -= Srow.max(1, keepdims=True)
                    Prow = np.exp(Srow)
                    ctx[rows] = (Prow @ Vd[h]) / Prow.sum(1, keepdims=True)
                out[:, 0, h * D : (h + 1) * D] = ctx
    if _trace:
        return out, res
    return out
